# revision 14
# baseline (speedup 1.0000x reference)
"""Causal self-attention kernel for 8 trn2 NeuronCores.

Sharding: 4 batches x 2 head-groups (8 heads each). Core c handles
batch c//2, heads (c%2)*8 .. (c%2)*8+8. Each core computes qkv for its
head-group, causal attention, and a partial projection; the host sums
the two head-group partials per batch and adds b_proj.

Schedule (v6): ScalarE runs ONLY the softmax exp (its pace limits the
attention inner loop); every other PSUM evacuation runs on VectorE.
The attention loop is software-pipelined (PV lags scores by 3 blocks)
and next-pair QK / V-tail / projection matmuls are statically
interleaved as PE filler so the PE never waits out the exp round-trip.
All inputs are bf16 (host-converted): same PE rate as fp32r but half
the DMA bytes and SBUF stream traffic. PSUM accumulation stays fp32.
"""

import sys
import os
from collections import deque

for _p in ("/opt/trn_rl_repo", "/root/.axon_site/_ro/trn_rl_repo"):
    if os.path.isdir(_p) and _p not in sys.path:
        sys.path.insert(0, _p)

import numpy as np
from ml_dtypes import bfloat16 as np_bf16
import concourse.bass as bass  # noqa: F401
import concourse.mybir as mybir
import concourse.tile as tile
from concourse import bacc, bass_utils

F32 = mybir.dt.float32
BF16 = mybir.dt.bfloat16
ActF = mybir.ActivationFunctionType

B, S, D, H = 4, 2048, 1024, 16
NH = 8          # heads per core
HPAIRS = NH // 2
KT = D // 128   # 8 k-tiles over D
N_CORES = 8

_nc_cache = {}


def build_nc(S_tok=S, n_cores=N_CORES):
    key = (S_tok, n_cores)
    if key in _nc_cache:
        return _nc_cache[key]
    IC = S_tok // 512      # query chunks per pair
    NT = S_tok // 128      # token tiles
    nc = bacc.Bacc("TRN2", target_bir_lowering=False, debug=False,
                   num_devices=n_cores)
    xT = nc.dram_tensor("xT", [D, S_tok], BF16, kind="ExternalInput").ap()
    Wq = nc.dram_tensor("Wq", [D + 1, 512], BF16, kind="ExternalInput").ap()
    Wk = nc.dram_tensor("Wk", [D + 1, 512], BF16, kind="ExternalInput").ap()
    Wv = nc.dram_tensor("Wv", [D + 1, 512], BF16, kind="ExternalInput").ap()
    Wp = nc.dram_tensor("Wp", [512, D], BF16, kind="ExternalInput").ap()
    out = nc.dram_tensor("out", [S_tok, D], F32, kind="ExternalOutput").ap()

    def dram_kpc(t, rows, cols):
        """[rows*128, cols] dram view -> [128, rows, cols] AP."""
        return t[0:rows * 128, cols].rearrange("(k p) c -> p k c", p=128)

    with tile.TileContext(nc) as tc:
        with tc.tile_pool(name="persist", bufs=1) as pp, \
             tc.tile_pool(name="hsb", bufs=1) as hsb, \
             tc.tile_pool(name="wqk", bufs=1) as wqk, \
             tc.tile_pool(name="ps", bufs=1, space="PSUM") as ps:
            # resident xT as one [128, k, S] bf16 tile
            xbig = pp.tile([128, KT, S_tok], BF16, name="xbig")
            xtr = [xbig[:, k, :] for k in range(KT)]
            # v in natural layout, 65-stride per head (64 v cols + ones col)
            v_sb = [pp.tile([128, 8 * 65], BF16, name=f"vsb{t}")
                    for t in range(NT)]
            # yT accumulation per head pair [local d, tokens], bf16
            yT = [pp.tile([128, S_tok], BF16, name=f"ytr{h}")
                  for h in range(HPAIRS)]
            # projection weights, prefetched mid-flight
            wpa = pp.tile([128, HPAIRS, D], BF16, name="wpa")
            wp = [wpa[:, k, :] for k in range(HPAIRS)]
            # triangle mask for the diagonal 128-strip:
            # mask[p, y] = 1 if y >= p else 0
            tri = pp.tile([128, 128], BF16, name="tri")
            nc.gpsimd.memset(tri, 1.0)
            nc.gpsimd.affine_select(
                out=tri, in_=tri, compare_op=mybir.AluOpType.is_ge,
                fill=0.0, base=0, pattern=[[1, 128]],
                channel_multiplier=-1)
            ones8 = pp.tile([128, 8, 1], BF16, name="ones8")
            nc.gpsimd.memset(ones8, 1.0)

            # ---- input DMAs (few, large, priority-ordered) ----
            def fetch_w(hp):
                """Queue the DMAs for head-pair hp's Q/K weights."""
                hs = slice(hp * 128, (hp + 1) * 128)
                wqa = wqk.tile([128, KT, 128], BF16, tag="wq", bufs=2,
                               name="wqa")
                nc.sync.dma_start(wqa, dram_kpc(Wq, KT, hs))
                wka = wqk.tile([128, KT, 128], BF16, tag="wk", bufs=2,
                               name="wka")
                nc.sync.dma_start(wka, dram_kpc(Wk, KT, hs))
                w9b = wqk.tile([128, 2], BF16, tag="w9b", bufs=2, name="w9b")
                nc.sync.dma_start(w9b[:, 0:1], Wq[D:D + 1, hs])
                nc.sync.dma_start(w9b[:, 1:2], Wk[D:D + 1, hs])
                w9f = wqk.tile([128, 2], F32, tag="w9f", bufs=2, name="w9f")
                nc.vector.tensor_copy(w9f, w9b)
                return ([wqa[:, k, :] for k in range(KT)],
                        [wka[:, k, :] for k in range(KT)],
                        w9f[:, 0:1], w9f[:, 1:2])

            # x chunk 0 + pair-0 weights first, interleaved by priority
            hs0 = slice(0, 128)
            wqa0 = wqk.tile([128, KT, 128], BF16, tag="wq", bufs=2,
                            name="wqa0")
            nc.sync.dma_start(wqa0, dram_kpc(Wq, KT, hs0))
            nc.sync.dma_start(xbig[:, 0:4, 0:512],
                              dram_kpc(xT, KT, slice(0, 512))[:, 0:4, :])
            nc.sync.dma_start(xbig[:, 4:KT, 0:512],
                              dram_kpc(xT, KT, slice(0, 512))[:, 4:KT, :])
            wka0 = wqk.tile([128, KT, 128], BF16, tag="wk", bufs=2,
                            name="wka0")
            nc.sync.dma_start(wka0, dram_kpc(Wk, KT, hs0))
            w9b0 = wqk.tile([128, 2], BF16, tag="w9b", bufs=2, name="w9b0")
            nc.sync.dma_start(w9b0[:, 0:1], Wq[D:D + 1, hs0])
            nc.sync.dma_start(w9b0[:, 1:2], Wk[D:D + 1, hs0])
            w9f0 = wqk.tile([128, 2], F32, tag="w9f", bufs=2, name="w9f0")
            nc.vector.tensor_copy(w9f0, w9b0)
            w_pair0 = ([wqa0[:, k, :] for k in range(KT)],
                       [wka0[:, k, :] for k in range(KT)],
                       w9f0[:, 0:1], w9f0[:, 1:2])
            # V weights (needed from the first V unit, after QK chunk 0)
            wvp_cm = tc.tile_pool(name="wvp", bufs=1)
            wvp = wvp_cm.__enter__()
            wva = wvp.tile([128, KT, 512], BF16, name="wva")
            nc.sync.dma_start(wva, dram_kpc(Wv, KT, slice(0, 512)))
            wv = [wva[:, k, :] for k in range(KT)]
            bvr = wvp.tile([1, 512], BF16, name="bvr")
            nc.sync.dma_start(bvr, Wv[D:D + 1, :])
            bvb = wvp.tile([128, 512], BF16, name="bvb")
            nc.gpsimd.partition_broadcast(bvb, bvr)
            # remaining x column chunks
            for c in range(1, S_tok // 512):
                cs = slice(c * 512, (c + 1) * 512)
                nc.sync.dma_start(xbig[:, :, cs], dram_kpc(xT, KT, cs))

            # ---- engine-op emitters ----
            def gen_qk_chunk(dst, w, w9, c):
                """One 512-token QK chunk: 8 accum matmuls + DVE evac."""
                psq = ps.tile([128, 512], F32, tag="aux", bufs=2, name="psq")
                for k in range(KT):
                    nc.tensor.matmul(
                        psq, w[k], xtr[k][:, c * 512:(c + 1) * 512],
                        start=(k == 0), stop=(k == KT - 1))
                    if k == 3:
                        yield
                nc.vector.tensor_scalar_add(
                    dst[:, c * 512:(c + 1) * 512], psq, w9)
                yield

            def emit_qk_chunk(dst, w, w9, c):
                for _ in gen_qk_chunk(dst, w, w9, c):
                    pass

            def emit_v_unit(t):
                """One token tile of V: 8 accum matmuls + bias/ones evac."""
                psv = ps.tile([128, 512], F32, tag="aux", bufs=2, name="psv")
                for k in range(KT):
                    nc.tensor.matmul(
                        psv, xtr[k][:, t * 128:(t + 1) * 128], wv[k],
                        start=(k == 0), stop=(k == KT - 1))
                vv = v_sb[t].rearrange("p (h c) -> p h c", c=65)
                nc.vector.tensor_add(
                    vv[:, :, 0:64],
                    psv.rearrange("p (h c) -> p h c", c=64),
                    bvb.rearrange("p (h c) -> p h c", c=64))
                nc.vector.tensor_copy(vv[:, :, 64:65], ones8)

            # filler machinery: generators that emit a sub-unit per next()
            fillers = deque()

            def emit_filler(n=1):
                for _ in range(n):
                    while fillers:
                        try:
                            next(fillers[0])
                            break
                        except StopIteration:
                            fillers.popleft()
                    else:
                        return

            def gen_v_range(t0, t1):
                for t in range(t0, t1):
                    psv = ps.tile([128, 512], F32, tag="aux", bufs=2,
                                  name="psv")
                    for k in range(KT):
                        nc.tensor.matmul(
                            psv, xtr[k][:, t * 128:(t + 1) * 128], wv[k],
                            start=(k == 0), stop=(k == KT - 1))
                        if k == 3:
                            yield
                    vv = v_sb[t].rearrange("p (h c) -> p h c", c=65)
                    nc.vector.tensor_add(
                        vv[:, :, 0:64],
                        psv.rearrange("p (h c) -> p h c", c=64),
                        bvb.rearrange("p (h c) -> p h c", c=64))
                    nc.vector.tensor_copy(vv[:, :, 64:65], ones8)
                    yield

            def gen_qk_pair(wq, wk, wq9, wk9, qt, kt_t):
                for c in range(IC):
                    yield from gen_qk_chunk(qt, wq, wq9, c)
                    yield from gen_qk_chunk(kt_t, wk, wk9, c)

            ppsb = {}

            def gen_proj_partial(tt):
                for nch in range(2):
                    pso = ps.tile([128, 512], F32, tag="aux", bufs=2,
                                  name="psop")
                    for k in range(HPAIRS - 1):
                        nc.tensor.matmul(
                            pso, yT[k][:, tt * 128:(tt + 1) * 128],
                            wp[k][:, nch * 512:(nch + 1) * 512],
                            start=(k == 0), stop=(k == HPAIRS - 2))
                    pt = hsb.tile([128, 512], F32, tag=f"pp{tt % 4}_{nch}",
                                  bufs=1, name="pt")
                    nc.vector.tensor_copy(pt, pso)
                    ppsb[(tt, nch)] = pt
                    yield

            def gen_proj_final(tt):
                k = HPAIRS - 1
                for nch in range(2):
                    pso = ps.tile([128, 512], F32, tag="aux", bufs=2,
                                  name="psof")
                    nc.tensor.matmul(
                        pso, yT[k][:, tt * 128:(tt + 1) * 128],
                        wp[k][:, nch * 512:(nch + 1) * 512],
                        start=True, stop=True)
                    ot = hsb.tile([128, 512], F32, tag="ot", bufs=4,
                                  name="ot")
                    nc.vector.tensor_add(ot, pso, ppsb.pop((tt, nch)))
                    nc.sync.dma_start(
                        out[tt * 128:(tt + 1) * 128,
                            nch * 512:(nch + 1) * 512], ot)
                    yield

            def gen_proj_tile(tt):
                for nch in range(2):
                    pso = ps.tile([128, 512], F32, tag="aux", bufs=2,
                                  name="pso")
                    for k in range(HPAIRS):
                        nc.tensor.matmul(
                            pso, yT[k][:, tt * 128:(tt + 1) * 128],
                            wp[k][:, nch * 512:(nch + 1) * 512],
                            start=(k == 0), stop=(k == HPAIRS - 1))
                        if k == 1:
                            yield
                    ot = hsb.tile([128, 512], F32, tag="ot", bufs=4,
                                  name="ot")
                    nc.vector.tensor_copy(ot, pso)
                    nc.sync.dma_start(
                        out[tt * 128:(tt + 1) * 128,
                            nch * 512:(nch + 1) * 512], ot)
                    yield

            # ---- head: QK pair 0 + V tiles 0..7, DMA-paced ----
            qkt = {}

            def qk_tiles(hp):
                qt = hsb.tile([128, S_tok], BF16, tag="qt", bufs=2, name="qt")
                kt_t = hsb.tile([128, S_tok], BF16, tag="kt", bufs=2,
                                name="kt")
                qkt[hp] = (qt, kt_t)
                return qt, kt_t

            qt0, kt0 = qk_tiles(0)
            wq, wk, wq9, wk9 = w_pair0
            for c in range(IC):
                emit_qk_chunk(qt0, wq, wq9, c)
                emit_qk_chunk(kt0, wk, wk9, c)
                if c > 0:
                    emit_v_unit(2 * (c - 1))
                    emit_v_unit(2 * c - 1)
            emit_v_unit(6)
            emit_v_unit(7)

            # ---- pair loop: pipelined attention with fillers ----
            for hp in range(HPAIRS):
                qt, kt_t = qkt[hp]
                if hp == 0:
                    fillers.append(gen_v_range(8, NT))
                if hp == 1:
                    nc.sync.dma_start(wpa, dram_kpc(Wp, HPAIRS, slice(0, D)))
                if hp + 1 < HPAIRS:
                    wnext = fetch_w(hp + 1)
                    qtn, ktn = qk_tiles(hp + 1)
                    fillers.append(gen_qk_pair(*wnext, qtn, ktn))
                if hp == HPAIRS - 1:
                    for tt in range(4 * (IC - 1), NT):
                        fillers.append(gen_proj_partial(tt))

                blocks = [(ic, jt) for ic in range(IC)
                          for jt in range(4 * ic + 4)]
                psys = {}
                ets = {}

                def emit_scores(n):
                    ic, jt = blocks[n]
                    tdx = jt - 4 * ic
                    off = 128 * tdx if tdx > 0 else 0
                    pss = ps.tile([128, 1024], F32, tag="pss", bufs=2,
                                  name="pss")
                    nc.tensor.matmul(
                        pss[:, off:512],
                        kt_t[0:64, jt * 128:(jt + 1) * 128],
                        qt[0:64, ic * 512 + off:(ic + 1) * 512],
                        start=True, stop=True, tile_position=(0, 0))
                    nc.tensor.matmul(
                        pss[:, 512 + off:1024],
                        kt_t[64:128, jt * 128:(jt + 1) * 128],
                        qt[64:128, ic * 512 + off:(ic + 1) * 512],
                        start=True, stop=True, tile_position=(64, 0))
                    et = hsb.tile([128, 1024], BF16, tag="et", bufs=6,
                                  name="et")
                    if off:
                        e2 = et.rearrange("p (h q) -> p h q", h=2)
                        p2 = pss.rearrange("p (h q) -> p h q", h=2)
                        nc.scalar.activation(e2[:, :, off:512],
                                             p2[:, :, off:512],
                                             ActF.Exp, scale=0.125)
                    else:
                        nc.scalar.activation(et, pss, ActF.Exp, scale=0.125)
                    if tdx >= 0:
                        nc.vector.tensor_mul(
                            et[:, off:off + 128], et[:, off:off + 128], tri)
                        nc.vector.tensor_mul(
                            et[:, 512 + off:512 + off + 128],
                            et[:, 512 + off:512 + off + 128], tri)
                    ets[n] = et

                def emit_pv(n):
                    ic, jt = blocks[n]
                    if jt == 0:
                        psys[ic] = (
                            ps.tile([65, 512], F32, tag="psyA", bufs=1,
                                    name="psyA"),
                            ps.tile([65, 512], F32, tag="psyB", bufs=1,
                                    name="psyB"))
                    et = ets.pop(n)
                    tdx = jt - 4 * ic
                    off = 128 * tdx if tdx > 0 else 0
                    for head in range(2):
                        vsl = v_sb[jt][:, (2 * hp + head) * 65:
                                       (2 * hp + head) * 65 + 65]
                        nc.tensor.matmul(
                            psys[ic][head][:, off:512], vsl,
                            et[:, head * 512 + off:(head + 1) * 512],
                            start=(jt == 0), stop=(jt == 4 * ic + 3))
                    if jt == 4 * ic + 3:
                        emit_ic_tail(ic)

                def emit_ic_tail(ic):
                    # PSUM evacuation (bf16 for yT) + SBUF-only normalize
                    sl = slice(ic * 512, (ic + 1) * 512)
                    zc = hsb.tile([1, 1024], BF16, tag="zc", bufs=1,
                                  name="zc")
                    for head in range(2):
                        t65 = hsb.tile([65, 512], BF16, tag="t65",
                                       bufs=2, name="t65")
                        nc.vector.tensor_copy(t65, psys[ic][head])
                        nc.sync.dma_start(
                            yT[hp][head * 64:(head + 1) * 64, sl],
                            t65[0:64, :])
                        nc.sync.dma_start(
                            zc[0:1, head * 512:(head + 1) * 512],
                            t65[64:65, :])
                    # reciprocal at full lane width: scatter the 1024 Z
                    # values over 128 partitions, recip in f32, gather back
                    zsb = hsb.tile([128, 8], BF16, tag="zsb", bufs=2,
                                   name="zsb")
                    nc.sync.dma_start(zsb, zc)
                    zs = hsb.tile([128, 8], F32, tag="zs", bufs=2,
                                  name="zs")
                    nc.vector.tensor_copy(zs, zsb)
                    nc.vector.reciprocal(zs, zs)
                    nc.vector.tensor_copy(zsb, zs)
                    nc.sync.dma_start(zc, zsb)
                    bcf = hsb.tile([128, 512], BF16, tag="bcf", bufs=2,
                                   name="bcf")
                    nc.gpsimd.partition_broadcast(bcf, zc[0:1, 512:1024])
                    nc.gpsimd.partition_broadcast(bcf[0:64, :],
                                                  zc[0:1, 0:512])
                    if hp == HPAIRS - 1 and ic == IC - 1:
                        for q in range(4):
                            qsl = slice(ic * 512 + q * 128,
                                        ic * 512 + (q + 1) * 128)
                            nc.vector.tensor_mul(
                                yT[hp][:, qsl], yT[hp][:, qsl],
                                bcf[:, q * 128:(q + 1) * 128])
                    else:
                        nc.vector.tensor_mul(yT[hp][:, sl], yT[hp][:, sl],
                                             bcf)
                    if hp == HPAIRS - 1 and ic > 0:
                        for tt in range(4 * (ic - 1), 4 * ic):
                            fillers.append(gen_proj_tile(tt))

                # software pipeline: PV lags scores by 4 blocks
                LAG = 4
                nb = len(blocks)
                for n in range(nb + LAG):
                    if n < nb:
                        emit_scores(n)
                    if n >= LAG:
                        emit_pv(n - LAG)
                    emit_filler(2 if (hp == HPAIRS - 1 and n >= 10) else 1)
            # drain remaining fillers (projection tail)
            for tt in range(4 * (IC - 1), NT):
                fillers.append(gen_proj_final(tt))
            emit_filler(1000)
            wvp_cm.__exit__(None, None, None)
    nc.finalize()
    _nc_cache[key] = nc
    return nc


def make_in_maps(x, W_attn, b_attn, W_proj):
    """Build per-core input dicts from full inputs (bf16 on the wire)."""
    Bx, Sx, Dx = x.shape
    in_maps = []
    for c in range(N_CORES):
        b = c // 2
        g = c % 2
        cs = slice(g * 512, (g + 1) * 512)
        xT_aug = np.ascontiguousarray(x[b].T)
        wq = np.concatenate([W_attn[:, 0:D][:, cs],
                             b_attn[0:D][cs][None, :]], axis=0)
        wk = np.concatenate([W_attn[:, D:2 * D][:, cs],
                             b_attn[D:2 * D][cs][None, :]], axis=0)
        wv = np.concatenate([W_attn[:, 2 * D:3 * D][:, cs],
                             b_attn[2 * D:3 * D][cs][None, :]], axis=0)
        wp = np.ascontiguousarray(W_proj[cs, :])
        in_maps.append({
            "xT": np.ascontiguousarray(xT_aug).astype(np_bf16),
            "Wq": np.ascontiguousarray(wq).astype(np_bf16),
            "Wk": np.ascontiguousarray(wk).astype(np_bf16),
            "Wv": np.ascontiguousarray(wv).astype(np_bf16),
            "Wp": wp.astype(np_bf16),
        })
    return in_maps


def kernel(x, W_attn, b_attn, W_proj, b_proj, trace=False):
    x = np.asarray(x, dtype=np.float32)
    W_attn = np.asarray(W_attn, dtype=np.float32)
    b_attn = np.asarray(b_attn, dtype=np.float32)
    W_proj = np.asarray(W_proj, dtype=np.float32)
    b_proj = np.asarray(b_proj, dtype=np.float32)
    nc = build_nc(x.shape[1], N_CORES)
    in_maps = make_in_maps(x, W_attn, b_attn, W_proj)
    res = bass_utils.run_bass_kernel_spmd(
        nc, in_maps, core_ids=list(range(N_CORES)), trace=trace)
    Bx, Sx, Dx = x.shape
    outp = np.empty((Bx, Sx, Dx), dtype=np.float32)
    for b in range(Bx):
        outp[b] = (res.results[2 * b]["out"] + res.results[2 * b + 1]["out"]
                   + b_proj[None, :])
    if trace:
        return outp, res
    return outp


# revision 15
# speedup vs baseline: 1.0113x; 1.0113x over previous
"""Causal self-attention kernel for 8 trn2 NeuronCores.

Sharding: 4 batches x 2 head-groups (8 heads each). Core c handles
batch c//2, heads (c%2)*8 .. (c%2)*8+8. Each core computes qkv for its
head-group, causal attention, and a partial projection; the host sums
the two head-group partials per batch and adds b_proj.

Schedule (v6): ScalarE runs ONLY the softmax exp (its pace limits the
attention inner loop); every other PSUM evacuation runs on VectorE.
The attention loop is software-pipelined (PV lags scores by 3 blocks)
and next-pair QK / V-tail / projection matmuls are statically
interleaved as PE filler so the PE never waits out the exp round-trip.
All inputs are bf16 (host-converted): same PE rate as fp32r but half
the DMA bytes and SBUF stream traffic. PSUM accumulation stays fp32.
"""

import sys
import os
from collections import deque

for _p in ("/opt/trn_rl_repo", "/root/.axon_site/_ro/trn_rl_repo"):
    if os.path.isdir(_p) and _p not in sys.path:
        sys.path.insert(0, _p)

import numpy as np
from ml_dtypes import bfloat16 as np_bf16
import concourse.bass as bass  # noqa: F401
import concourse.mybir as mybir
import concourse.tile as tile
from concourse import bacc, bass_utils

F32 = mybir.dt.float32
BF16 = mybir.dt.bfloat16
ActF = mybir.ActivationFunctionType

B, S, D, H = 4, 2048, 1024, 16
NH = 8          # heads per core
HPAIRS = NH // 2
KT = D // 128   # 8 k-tiles over D
N_CORES = 8

_nc_cache = {}


def build_nc(S_tok=S, n_cores=N_CORES):
    key = (S_tok, n_cores)
    if key in _nc_cache:
        return _nc_cache[key]
    IC = S_tok // 512      # query chunks per pair
    NT = S_tok // 128      # token tiles
    nc = bacc.Bacc("TRN2", target_bir_lowering=False, debug=False,
                   num_devices=n_cores)
    xT = nc.dram_tensor("xT", [D, S_tok], BF16, kind="ExternalInput").ap()
    Wq = nc.dram_tensor("Wq", [D + 1, 512], BF16, kind="ExternalInput").ap()
    Wk = nc.dram_tensor("Wk", [D + 1, 512], BF16, kind="ExternalInput").ap()
    Wv = nc.dram_tensor("Wv", [D + 1, 512], BF16, kind="ExternalInput").ap()
    Wp = nc.dram_tensor("Wp", [512, D], BF16, kind="ExternalInput").ap()
    out = nc.dram_tensor("out", [S_tok, D], F32, kind="ExternalOutput").ap()

    def dram_kpc(t, rows, cols):
        """[rows*128, cols] dram view -> [128, rows, cols] AP."""
        return t[0:rows * 128, cols].rearrange("(k p) c -> p k c", p=128)

    with tile.TileContext(nc) as tc:
        with tc.tile_pool(name="persist", bufs=1) as pp, \
             tc.tile_pool(name="hsb", bufs=1) as hsb, \
             tc.tile_pool(name="wqk", bufs=1) as wqk, \
             tc.tile_pool(name="ps", bufs=1, space="PSUM") as ps:
            # resident xT as one [128, k, S] bf16 tile
            xbig = pp.tile([128, KT, S_tok], BF16, name="xbig")
            xtr = [xbig[:, k, :] for k in range(KT)]
            # v in natural layout, 65-stride per head (64 v cols + ones col)
            v_sb = [pp.tile([128, 8 * 65], BF16, name=f"vsb{t}")
                    for t in range(NT)]
            # yT accumulation per head pair [local d, tokens], bf16
            yT = [pp.tile([128, S_tok], BF16, name=f"ytr{h}")
                  for h in range(HPAIRS)]
            # projection weights, prefetched mid-flight
            wpa = pp.tile([128, HPAIRS, D], BF16, name="wpa")
            wp = [wpa[:, k, :] for k in range(HPAIRS)]
            # triangle mask for the diagonal 128-strip:
            # mask[p, y] = 1 if y >= p else 0
            tri = pp.tile([128, 128], BF16, name="tri")
            nc.gpsimd.memset(tri, 1.0)
            nc.gpsimd.affine_select(
                out=tri, in_=tri, compare_op=mybir.AluOpType.is_ge,
                fill=0.0, base=0, pattern=[[1, 128]],
                channel_multiplier=-1)
            ones8 = pp.tile([128, 8, 1], BF16, name="ones8")
            nc.gpsimd.memset(ones8, 1.0)

            # ---- input DMAs (few, large, priority-ordered) ----
            def fetch_w(hp):
                """Queue the DMAs for head-pair hp's Q/K weights."""
                hs = slice(hp * 128, (hp + 1) * 128)
                wqa = wqk.tile([128, KT, 128], BF16, tag="wq", bufs=2,
                               name="wqa")
                nc.sync.dma_start(wqa, dram_kpc(Wq, KT, hs))
                wka = wqk.tile([128, KT, 128], BF16, tag="wk", bufs=2,
                               name="wka")
                nc.sync.dma_start(wka, dram_kpc(Wk, KT, hs))
                w9b = wqk.tile([128, 2], BF16, tag="w9b", bufs=2, name="w9b")
                nc.sync.dma_start(w9b[:, 0:1], Wq[D:D + 1, hs])
                nc.sync.dma_start(w9b[:, 1:2], Wk[D:D + 1, hs])
                w9f = wqk.tile([128, 2], F32, tag="w9f", bufs=2, name="w9f")
                nc.vector.tensor_copy(w9f, w9b)
                return ([wqa[:, k, :] for k in range(KT)],
                        [wka[:, k, :] for k in range(KT)],
                        w9f[:, 0:1], w9f[:, 1:2])

            # x chunk 0 + pair-0 weights first, interleaved by priority
            hs0 = slice(0, 128)
            wqa0 = wqk.tile([128, KT, 128], BF16, tag="wq", bufs=2,
                            name="wqa0")
            nc.sync.dma_start(wqa0, dram_kpc(Wq, KT, hs0))
            nc.sync.dma_start(xbig[:, 0:4, 0:512],
                              dram_kpc(xT, KT, slice(0, 512))[:, 0:4, :])
            nc.sync.dma_start(xbig[:, 4:KT, 0:512],
                              dram_kpc(xT, KT, slice(0, 512))[:, 4:KT, :])
            wka0 = wqk.tile([128, KT, 128], BF16, tag="wk", bufs=2,
                            name="wka0")
            nc.sync.dma_start(wka0, dram_kpc(Wk, KT, hs0))
            w9b0 = wqk.tile([128, 2], BF16, tag="w9b", bufs=2, name="w9b0")
            nc.sync.dma_start(w9b0[:, 0:1], Wq[D:D + 1, hs0])
            nc.sync.dma_start(w9b0[:, 1:2], Wk[D:D + 1, hs0])
            w9f0 = wqk.tile([128, 2], F32, tag="w9f", bufs=2, name="w9f0")
            nc.vector.tensor_copy(w9f0, w9b0)
            w_pair0 = ([wqa0[:, k, :] for k in range(KT)],
                       [wka0[:, k, :] for k in range(KT)],
                       w9f0[:, 0:1], w9f0[:, 1:2])
            # V weights (needed from the first V unit, after QK chunk 0)
            wvp_cm = tc.tile_pool(name="wvp", bufs=1)
            wvp = wvp_cm.__enter__()
            wva = wvp.tile([128, KT, 512], BF16, name="wva")
            nc.sync.dma_start(wva, dram_kpc(Wv, KT, slice(0, 512)))
            wv = [wva[:, k, :] for k in range(KT)]
            bvr = wvp.tile([1, 512], BF16, name="bvr")
            nc.sync.dma_start(bvr, Wv[D:D + 1, :])
            bvb = wvp.tile([128, 512], BF16, name="bvb")
            nc.gpsimd.partition_broadcast(bvb, bvr)
            # remaining x column chunks
            for c in range(1, S_tok // 512):
                cs = slice(c * 512, (c + 1) * 512)
                nc.sync.dma_start(xbig[:, :, cs], dram_kpc(xT, KT, cs))

            # ---- engine-op emitters ----
            def gen_qk_chunk(dst, w, w9, c):
                """One 512-token QK chunk: 8 accum matmuls + DVE evac."""
                psq = ps.tile([128, 512], F32, tag="aux", bufs=2, name="psq")
                for k in range(KT):
                    nc.tensor.matmul(
                        psq, w[k], xtr[k][:, c * 512:(c + 1) * 512],
                        start=(k == 0), stop=(k == KT - 1))
                    if k == 3:
                        yield
                nc.vector.tensor_scalar_add(
                    dst[:, c * 512:(c + 1) * 512], psq, w9)
                yield

            def emit_qk_chunk(dst, w, w9, c):
                for _ in gen_qk_chunk(dst, w, w9, c):
                    pass

            def emit_v_unit(t):
                """One token tile of V: 8 accum matmuls + bias/ones evac."""
                psv = ps.tile([128, 512], F32, tag="aux", bufs=2, name="psv")
                for k in range(KT):
                    nc.tensor.matmul(
                        psv, xtr[k][:, t * 128:(t + 1) * 128], wv[k],
                        start=(k == 0), stop=(k == KT - 1))
                vv = v_sb[t].rearrange("p (h c) -> p h c", c=65)
                nc.vector.tensor_add(
                    vv[:, :, 0:64],
                    psv.rearrange("p (h c) -> p h c", c=64),
                    bvb.rearrange("p (h c) -> p h c", c=64))
                nc.vector.tensor_copy(vv[:, :, 64:65], ones8)

            # filler machinery: generators that emit a sub-unit per next()
            fillers = deque()

            def emit_filler(n=1):
                for _ in range(n):
                    while fillers:
                        try:
                            next(fillers[0])
                            break
                        except StopIteration:
                            fillers.popleft()
                    else:
                        return

            def gen_v_range(t0, t1):
                for t in range(t0, t1):
                    psv = ps.tile([128, 512], F32, tag="aux", bufs=2,
                                  name="psv")
                    for k in range(KT):
                        nc.tensor.matmul(
                            psv, xtr[k][:, t * 128:(t + 1) * 128], wv[k],
                            start=(k == 0), stop=(k == KT - 1))
                        if k == 3:
                            yield
                    vv = v_sb[t].rearrange("p (h c) -> p h c", c=65)
                    nc.vector.tensor_add(
                        vv[:, :, 0:64],
                        psv.rearrange("p (h c) -> p h c", c=64),
                        bvb.rearrange("p (h c) -> p h c", c=64))
                    nc.vector.tensor_copy(vv[:, :, 64:65], ones8)
                    yield

            def gen_qk_pair(wq, wk, wq9, wk9, qt, kt_t):
                for c in range(IC):
                    yield from gen_qk_chunk(qt, wq, wq9, c)
                    yield from gen_qk_chunk(kt_t, wk, wk9, c)

            ppsb = {}

            def gen_proj_partial(tt):
                for nch in range(2):
                    pso = ps.tile([128, 512], F32, tag="aux", bufs=2,
                                  name="psop")
                    for k in range(HPAIRS - 1):
                        nc.tensor.matmul(
                            pso, yT[k][:, tt * 128:(tt + 1) * 128],
                            wp[k][:, nch * 512:(nch + 1) * 512],
                            start=(k == 0), stop=(k == HPAIRS - 2))
                    pt = hsb.tile([128, 512], F32, tag=f"pp{tt % 4}_{nch}",
                                  bufs=1, name="pt")
                    nc.vector.tensor_copy(pt, pso)
                    ppsb[(tt, nch)] = pt
                    yield

            def gen_proj_final(tt):
                k = HPAIRS - 1
                for nch in range(2):
                    pso = ps.tile([128, 512], F32, tag="aux", bufs=2,
                                  name="psof")
                    nc.tensor.matmul(
                        pso, yT[k][:, tt * 128:(tt + 1) * 128],
                        wp[k][:, nch * 512:(nch + 1) * 512],
                        start=True, stop=True)
                    ot = hsb.tile([128, 512], F32, tag="ot", bufs=4,
                                  name="ot")
                    nc.vector.tensor_add(ot, pso, ppsb.pop((tt, nch)))
                    nc.sync.dma_start(
                        out[tt * 128:(tt + 1) * 128,
                            nch * 512:(nch + 1) * 512], ot)
                    yield

            def gen_proj_tile(tt, on_act=False):
                for nch in range(2):
                    pso = ps.tile([128, 512], F32, tag="aux", bufs=2,
                                  name="pso")
                    for k in range(HPAIRS):
                        nc.tensor.matmul(
                            pso, yT[k][:, tt * 128:(tt + 1) * 128],
                            wp[k][:, nch * 512:(nch + 1) * 512],
                            start=(k == 0), stop=(k == HPAIRS - 1))
                        if k == 1:
                            yield
                    ot = hsb.tile([128, 512], F32, tag="ot", bufs=4,
                                  name="ot")
                    if on_act:
                        nc.scalar.activation(ot, pso, ActF.Copy)
                    else:
                        nc.vector.tensor_copy(ot, pso)
                    nc.sync.dma_start(
                        out[tt * 128:(tt + 1) * 128,
                            nch * 512:(nch + 1) * 512], ot)
                    yield

            # ---- head: QK pair 0 + V tiles 0..7, DMA-paced ----
            qkt = {}

            def qk_tiles(hp):
                qt = hsb.tile([128, S_tok], BF16, tag="qt", bufs=2, name="qt")
                kt_t = hsb.tile([128, S_tok], BF16, tag="kt", bufs=2,
                                name="kt")
                qkt[hp] = (qt, kt_t)
                return qt, kt_t

            qt0, kt0 = qk_tiles(0)
            wq, wk, wq9, wk9 = w_pair0
            for c in range(IC):
                emit_qk_chunk(qt0, wq, wq9, c)
                emit_qk_chunk(kt0, wk, wk9, c)
                if c > 0:
                    emit_v_unit(2 * (c - 1))
                    emit_v_unit(2 * c - 1)
            emit_v_unit(6)
            emit_v_unit(7)

            # ---- pair loop: pipelined attention with fillers ----
            for hp in range(HPAIRS):
                qt, kt_t = qkt[hp]
                if hp == 0:
                    fillers.append(gen_v_range(8, NT))
                if hp == 1:
                    nc.sync.dma_start(wpa, dram_kpc(Wp, HPAIRS, slice(0, D)))
                if hp + 1 < HPAIRS:
                    wnext = fetch_w(hp + 1)
                    qtn, ktn = qk_tiles(hp + 1)
                    fillers.append(gen_qk_pair(*wnext, qtn, ktn))
                if hp == HPAIRS - 1:
                    for tt in range(4 * (IC - 1), NT):
                        fillers.append(gen_proj_partial(tt))

                blocks = [(ic, jt) for ic in range(IC)
                          for jt in range(4 * ic + 4)]
                psys = {}
                ets = {}

                def emit_scores(n):
                    ic, jt = blocks[n]
                    tdx = jt - 4 * ic
                    off = 128 * tdx if tdx > 0 else 0
                    pss = ps.tile([128, 1024], F32, tag="pss", bufs=2,
                                  name="pss")
                    nc.tensor.matmul(
                        pss[:, off:512],
                        kt_t[0:64, jt * 128:(jt + 1) * 128],
                        qt[0:64, ic * 512 + off:(ic + 1) * 512],
                        start=True, stop=True, tile_position=(0, 0))
                    nc.tensor.matmul(
                        pss[:, 512 + off:1024],
                        kt_t[64:128, jt * 128:(jt + 1) * 128],
                        qt[64:128, ic * 512 + off:(ic + 1) * 512],
                        start=True, stop=True, tile_position=(64, 0))
                    et = hsb.tile([128, 1024], BF16, tag="et", bufs=6,
                                  name="et")
                    if off:
                        e2 = et.rearrange("p (h q) -> p h q", h=2)
                        p2 = pss.rearrange("p (h q) -> p h q", h=2)
                        nc.scalar.activation(e2[:, :, off:512],
                                             p2[:, :, off:512],
                                             ActF.Exp, scale=0.125)
                    else:
                        nc.scalar.activation(et, pss, ActF.Exp, scale=0.125)
                    if tdx >= 0:
                        nc.vector.tensor_mul(
                            et[:, off:off + 128], et[:, off:off + 128], tri)
                        nc.vector.tensor_mul(
                            et[:, 512 + off:512 + off + 128],
                            et[:, 512 + off:512 + off + 128], tri)
                    ets[n] = et

                def emit_pv(n):
                    ic, jt = blocks[n]
                    if jt == 0:
                        psys[ic] = (
                            ps.tile([65, 512], F32, tag="psyA", bufs=1,
                                    name="psyA"),
                            ps.tile([65, 512], F32, tag="psyB", bufs=1,
                                    name="psyB"))
                    et = ets.pop(n)
                    tdx = jt - 4 * ic
                    off = 128 * tdx if tdx > 0 else 0
                    for head in range(2):
                        vsl = v_sb[jt][:, (2 * hp + head) * 65:
                                       (2 * hp + head) * 65 + 65]
                        nc.tensor.matmul(
                            psys[ic][head][:, off:512], vsl,
                            et[:, head * 512 + off:(head + 1) * 512],
                            start=(jt == 0), stop=(jt == 4 * ic + 3))
                    if jt == 4 * ic + 3:
                        emit_ic_tail(ic)

                def emit_ic_tail(ic):
                    # PSUM evacuation (bf16 for yT) + SBUF-only normalize
                    sl = slice(ic * 512, (ic + 1) * 512)
                    zc = hsb.tile([1, 1024], BF16, tag="zc", bufs=1,
                                  name="zc")
                    for head in range(2):
                        t65 = hsb.tile([65, 512], BF16, tag="t65",
                                       bufs=2, name="t65")
                        nc.vector.tensor_copy(t65, psys[ic][head])
                        nc.sync.dma_start(
                            yT[hp][head * 64:(head + 1) * 64, sl],
                            t65[0:64, :])
                        nc.sync.dma_start(
                            zc[0:1, head * 512:(head + 1) * 512],
                            t65[64:65, :])
                    # reciprocal at full lane width: scatter the 1024 Z
                    # values over 128 partitions, recip in f32, gather back
                    zsb = hsb.tile([128, 8], BF16, tag="zsb", bufs=2,
                                   name="zsb")
                    nc.sync.dma_start(zsb, zc)
                    zs = hsb.tile([128, 8], F32, tag="zs", bufs=2,
                                  name="zs")
                    nc.vector.tensor_copy(zs, zsb)
                    nc.vector.reciprocal(zs, zs)
                    nc.vector.tensor_copy(zsb, zs)
                    nc.sync.dma_start(zc, zsb)
                    bcf = hsb.tile([128, 512], BF16, tag="bcf", bufs=2,
                                   name="bcf")
                    nc.gpsimd.partition_broadcast(bcf, zc[0:1, 512:1024])
                    nc.gpsimd.partition_broadcast(bcf[0:64, :],
                                                  zc[0:1, 0:512])
                    if hp == HPAIRS - 1 and ic == IC - 1:
                        for q in range(4):
                            qsl = slice(ic * 512 + q * 128,
                                        ic * 512 + (q + 1) * 128)
                            nc.vector.tensor_mul(
                                yT[hp][:, qsl], yT[hp][:, qsl],
                                bcf[:, q * 128:(q + 1) * 128])
                    else:
                        nc.vector.tensor_mul(yT[hp][:, sl], yT[hp][:, sl],
                                             bcf)
                    if hp == HPAIRS - 1 and ic > 0:
                        for tt in range(4 * (ic - 1), 4 * ic):
                            fillers.append(
                                gen_proj_tile(tt, on_act=(ic == IC - 1)))

                # software pipeline: PV lags scores by 4 blocks
                LAG = 4
                nb = len(blocks)
                for n in range(nb + LAG):
                    if n < nb:
                        emit_scores(n)
                    if n >= LAG:
                        emit_pv(n - LAG)
                    emit_filler(2 if (hp == HPAIRS - 1 and n >= 10) else 1)
            # drain remaining fillers (projection tail)
            for tt in range(4 * (IC - 1), NT):
                fillers.append(gen_proj_final(tt))
            emit_filler(1000)
            wvp_cm.__exit__(None, None, None)
    nc.finalize()
    _nc_cache[key] = nc
    return nc


def make_in_maps(x, W_attn, b_attn, W_proj):
    """Build per-core input dicts from full inputs (bf16 on the wire)."""
    Bx, Sx, Dx = x.shape
    in_maps = []
    for c in range(N_CORES):
        b = c // 2
        g = c % 2
        cs = slice(g * 512, (g + 1) * 512)
        xT_aug = np.ascontiguousarray(x[b].T)
        wq = np.concatenate([W_attn[:, 0:D][:, cs],
                             b_attn[0:D][cs][None, :]], axis=0)
        wk = np.concatenate([W_attn[:, D:2 * D][:, cs],
                             b_attn[D:2 * D][cs][None, :]], axis=0)
        wv = np.concatenate([W_attn[:, 2 * D:3 * D][:, cs],
                             b_attn[2 * D:3 * D][cs][None, :]], axis=0)
        wp = np.ascontiguousarray(W_proj[cs, :])
        in_maps.append({
            "xT": np.ascontiguousarray(xT_aug).astype(np_bf16),
            "Wq": np.ascontiguousarray(wq).astype(np_bf16),
            "Wk": np.ascontiguousarray(wk).astype(np_bf16),
            "Wv": np.ascontiguousarray(wv).astype(np_bf16),
            "Wp": wp.astype(np_bf16),
        })
    return in_maps


def kernel(x, W_attn, b_attn, W_proj, b_proj, trace=False):
    x = np.asarray(x, dtype=np.float32)
    W_attn = np.asarray(W_attn, dtype=np.float32)
    b_attn = np.asarray(b_attn, dtype=np.float32)
    W_proj = np.asarray(W_proj, dtype=np.float32)
    b_proj = np.asarray(b_proj, dtype=np.float32)
    nc = build_nc(x.shape[1], N_CORES)
    in_maps = make_in_maps(x, W_attn, b_attn, W_proj)
    res = bass_utils.run_bass_kernel_spmd(
        nc, in_maps, core_ids=list(range(N_CORES)), trace=trace)
    Bx, Sx, Dx = x.shape
    outp = np.empty((Bx, Sx, Dx), dtype=np.float32)
    for b in range(Bx):
        outp[b] = (res.results[2 * b]["out"] + res.results[2 * b + 1]["out"]
                   + b_proj[None, :])
    if trace:
        return outp, res
    return outp
